# revision 2
# baseline (speedup 1.0000x reference)
"""DEQ block (Anderson acceleration, 6 iters, m=3) on 8 trn2 NeuronCores.

Data-parallel over batch: each core gets 512 of 4096 samples; W_z/W_x/b
replicated.  Per core the 512 samples are processed as two sequential
halves of 256 (2 m-tiles of 128) so all per-sample state stays SBUF
resident in fp32.  Matmuls run as float32r (FP22 reads, fp32 accumulate).

Per iteration i (z update, sample-major state):
  f   = tanh(z @ W_z + xwx)            PE (+identity-matmul xwx add) + ACT
  g   = f - z                          DVE scalar_tensor_tensor, in place
  u   = beta*g + z                     DVE scalar_tensor_tensor
  i<3:  z' = u  (buffer alias, no copy)
  i>=3: 2x2 regularized Anderson solve from 3 fresh dots
        P=<g,g> (ACT square+accum), Q1=<g,g1>, Q2=<g,g2> (DVE TTR),
        gram history terms reused from previous iterations' P/Q1;
        z' = s0*u + gamma1*u1 + gamma2*u2  (ACT scale + 2 DVE STT)
"""

import sys

sys.path.insert(0, "/opt/trn_rl_repo")

import numpy as np
from contextlib import ExitStack

import concourse.bass as bass
import concourse.tile as tile
from concourse import bacc, mybir, masks
from concourse import bass_utils

F32 = mybir.dt.float32
F32R = mybir.dt.float32r
F16 = mybir.dt.float16
ALU = mybir.AluOpType
ACTF = mybir.ActivationFunctionType

B, D = 4096, 2048
NCORES = 8
BC = B // NCORES          # 512 samples per core
NHALF = 2                 # sequential halves per core
CH = BC // NHALF          # 256 samples per half
MT = CH // 128            # 2 m-tiles per half
KT = D // 128             # 16 k-tiles
NT = D // 512             # 4 n-slices
RWZ = 4                   # W_z k-tiles kept SBUF resident; rest streamed
MAX_ITER, MAND = 6, 3
BETA, LAM = 0.8, 1e-4

_CACHE = {}

import os
NITER = int(os.environ.get("K_NITER", str(MAX_ITER)))   # iterations per half
NHALVES = int(os.environ.get("K_NHALVES", "2"))
FAKE_RES = int(os.environ.get("K_FAKE_RESIDENT", "0"))  # timing expt: no W stream


def _r(ap):
    return ap.bitcast(F32R)


def _build():
    nc = bacc.Bacc("TRN2", target_bir_lowering=False, debug=False,
                   num_devices=NCORES)

    x_d = nc.dram_tensor("x", [BC, D], F32, kind="ExternalInput").ap()
    wz_d = nc.dram_tensor("W_z", [D, D], F32, kind="ExternalInput").ap()
    wx_d = nc.dram_tensor("W_x", [D, D], F32, kind="ExternalInput").ap()
    b_d = nc.dram_tensor("b", [D], F32, kind="ExternalInput").ap()
    out_d = nc.dram_tensor("z_out", [BC, D], F32, kind="ExternalOutput").ap()
    # staging for half-1's xwx (computed in phase 0, reloaded at half 1)
    xwx1_d = nc.dram_tensor("xwx1_stage", [MT, 128, D], F16, kind="Internal").ap()

    with tile.TileContext(nc) as tc, ExitStack() as ctx:
        # ---------------- pools ----------------
        state = ctx.enter_context(tc.tile_pool(name="state", bufs=1))

        def persist(shape, nm):
            return state.tile(shape, F32, tag=nm, name=nm)

        wz16 = [state.tile([128, D], F16, tag=f"wz16_{k}", name=f"wz16_{k}")
                for k in range(KT)]
        zbuf = [persist([128, D], f"zbuf{m}") for m in range(MT)]
        gsl = [[persist([128, D], f"g{j}_{m}") for m in range(MT)]
               for j in range(3)]
        usl = [[persist([128, D], f"u{j}_{m}") for m in range(MT)]
               for j in range(3)]
        xwx = [state.tile([128, D], F16, tag=f"xwx{m}", name=f"xwx{m}")
               for m in range(MT)]
        ident = persist([128, 128], "ident")

        wpool = ctx.enter_context(tc.tile_pool(name="wstream", bufs=2))
        ztpool = ctx.enter_context(tc.tile_pool(name="ztp", bufs=33))
        dots = ctx.enter_context(tc.tile_pool(name="dots", bufs=40))
        typs = ctx.enter_context(tc.tile_pool(name="tpsum", bufs=3, space="PSUM"))
        yps = ctx.enter_context(tc.tile_pool(name="ypsum", bufs=4, space="PSUM"))

        pdump = state.tile([128, 512], F32, tag="pdump", name="pdump")
        qdump = state.tile([128, 512], F32, tag="qdump", name="qdump")
        masks.make_identity(nc, ident[:])
        identh = state.tile([128, 128], F16, tag="identh", name="identh")
        nc.vector.tensor_copy(identh[:], ident[:])
        rid = ident[:]          # fp32, rhs of fp32 transposes
        ridh = identh[:]        # fp16, lhsT of the xwx identity-matmul

        # W_z: DMA fp32 rows in, round to resident fp16 tiles on DVE
        for k in range(KT):
            for j in range(2):
                wrow = wpool.tile([128, 1024], F32, tag="w", name=f"wl{k}_{j}")
                nc.sync.dma_start(wrow[:], wz_d[k * 128:(k + 1) * 128,
                                               j * 1024:(j + 1) * 1024])
                nc.vector.tensor_copy(wz16[k][:, j * 1024:(j + 1) * 1024],
                                      wrow[:])

        def uw(j, ap):
            # usl[0]/usl[1] memlocs are fp32r-consumed (XT backing): every
            # engine write into them must round to fp32r for the verifier
            return _r(ap) if j in (0, 1) else ap

        def stt(out, in0, scalar, in1, op0, op1):
            nc.vector.scalar_tensor_tensor(
                out=out, in0=in0, scalar=scalar, in1=in1, op0=op0, op1=op1)

        # XT backing: 16 transposed-x k-rows [128, 512] live inside the
        # (not yet used) u-ring tiles during phase 0.
        def xt_sl(k, q):
            back = [usl[0][0], usl[0][1], usl[1][0], usl[1][1]][k // 4]
            off = (k % 4) * 512 + q * 128
            return back[:, off:off + 128]

        # ---------------- phase 0: xwx for all 4 quarter-tiles ----------------
        for q in range(4):
            xs = []
            for h2 in range(2):
                xst = wpool.tile([128, 1024], F32, tag="w", name=f"xst{q}_{h2}")
                nc.sync.dma_start(xst[:], x_d[q * 128:(q + 1) * 128,
                                               h2 * 1024:(h2 + 1) * 1024])
                xs.append(xst)
            for k in range(KT):
                tp = typs.tile([128, 128], F32, tag="tp", name=f"xtp{q}_{k}")
                src = xs[k // 8][:, (k % 8) * 128:(k % 8 + 1) * 128]
                nc.tensor.transpose(tp[:], src, rid)
                nc.scalar.copy(_r(xt_sl(k, q)), tp[:])


        b2d = b_d.rearrange("(p n) -> p n", p=1)
        for n in range(NT):
            b1 = wpool.tile([1, 512], F32, tag="w", name=f"b1_{n}")
            nc.sync.dma_start(b1[:], b2d[:, n * 512:(n + 1) * 512])
            bsl = wpool.tile([128, 512], F32, tag="w", name=f"bsl{n}")
            nc.gpsimd.partition_broadcast(bsl[:], b1[:])
            ps = [yps.tile([128, 512], F32, tag="yp", name=f"xwps{n}_{q}") for q in range(4)]
            for k in range(KT):
                wt = wpool.tile([128, 512], F32R, tag="w", name=f"wx{n}_{k}")
                nc.sync.dma_start(wt[:], _r(wx_d[k * 128:(k + 1) * 128,
                                                 n * 512:(n + 1) * 512]))
                for q in range(4):
                    nc.tensor.matmul(ps[q][:], _r(xt_sl(k, q)), wt[:],
                                     start=(k == 0), stop=(k == KT - 1))
            for q in range(4):
                if q < MT:
                    dst = xwx[q][:, n * 512:(n + 1) * 512]
                else:
                    dst = zbuf[q - MT].bitcast(F16)[:, n * 512:(n + 1) * 512]
                stt(dst, ps[q][:], 1.0, bsl[:], ALU.mult, ALU.add)
        for m in range(MT):
            nc.sync.dma_start(xwx1_d[m],
                              zbuf[m].bitcast(F16)[:, 0:D])

        # ---------------- per-half iterations ----------------
        def emit_half(h):
            if h == 1:
                for m in range(MT):
                    nc.sync.dma_start(xwx[m][:], xwx1_d[m])

            hist = {}  # (kind, i, m) -> [128,1] ap

            # iteration 0: z=0 -> g0 = tanh(xwx), u0 = beta*g0, z1 aliases u0
            for m in range(MT):
                nc.scalar.activation(gsl[0][m][:], xwx[m][:], ACTF.Tanh)
                nc.vector.tensor_scalar_mul(_r(usl[0][m][:]), gsl[0][m][:], BETA)

            for i in range(1, NITER):
                gi, ui = gsl[i % 3], usl[i % 3]
                g1, g2 = gsl[(i - 1) % 3], gsl[(i - 2) % 3]
                u1, u2 = usl[(i - 1) % 3], usl[(i - 2) % 3]
                zc = usl[i - 1] if i <= 3 else zbuf  # current z (alias)

                # transpose z into lhsT k-tiles
                zt = {}
                for m in range(MT):
                    for k in range(KT):
                        tp = typs.tile([128, 128], F32, tag="tp", name=f"tp{h}_{i}_{m}_{k}")
                        nc.tensor.transpose(
                            tp[:], zc[m][:, k * 128:(k + 1) * 128], rid)
                        zs = ztpool.tile([128, 128], F16, tag="zt",
                                         name=f"zt{h}_{i}_{m}_{k}")
                        nc.scalar.copy(zs[:], tp[:])
                        zt[m, k] = zs

                # matmul + xwx add + tanh, n-slice major
                for n in range(NT):
                    ps = [yps.tile([128, 512], F32, tag="yp", name=f"yp{h}_{i}_{n}_{m}")
                          for m in range(MT)]
                    for k in range(KT):
                        wsl = wz16[k][:, n * 512:(n + 1) * 512]
                        for m in range(MT):
                            nc.tensor.matmul(ps[m][:], zt[m, k][:], wsl,
                                             start=(k == 0), stop=False)
                    for m in range(MT):
                        nc.tensor.matmul(ps[m][:], ridh,
                                         xwx[m][:, n * 512:(n + 1) * 512],
                                         start=False, stop=True)
                        nc.scalar.activation(gi[m][:, n * 512:(n + 1) * 512],
                                             ps[m][:], ACTF.Tanh)

                for m in range(MT):
                    # g = f - z ; u = beta*g + z
                    stt(gi[m][:], gi[m][:], 1.0, zc[m][:], ALU.mult, ALU.subtract)
                    stt(uw(i % 3, ui[m][:]), gi[m][:], BETA, zc[m][:], ALU.mult, ALU.add)

                    # P = <g,g> on ACT (square + accum), dumped to PSUM
                    pc = dots.tile([128, 4], F32, tag="d", name=f"pc{h}_{i}_{m}")
                    for c in range(4):
                        nc.scalar.activation(pdump[:],
                                             gi[m][:, c * 512:(c + 1) * 512],
                                             ACTF.Square,
                                             accum_out=pc[:, c:c + 1])
                    pp = dots.tile([128, 1], F32, tag="d", name=f"p{h}_{i}_{m}")
                    nc.vector.tensor_reduce(pp[:], pc[:], mybir.AxisListType.X,
                                            ALU.add)
                    hist["P", i, m] = pp

                    def ttr_dot(gh, nm):
                        qc = dots.tile([128, 4], F32, tag="d", name=f"{nm}c")
                        for c in range(4):
                            nc.vector.scalar_tensor_tensor(
                                out=qdump[:],
                                in0=gi[m][:, c * 512:(c + 1) * 512],
                                scalar=1.0,
                                in1=gh[m][:, c * 512:(c + 1) * 512],
                                op0=ALU.mult, op1=ALU.mult,
                                accum_out=qc[:, c:c + 1])
                        qq = dots.tile([128, 1], F32, tag="d", name=nm)
                        nc.vector.tensor_reduce(qq[:], qc[:],
                                                mybir.AxisListType.X, ALU.add)
                        return qq

                    if i >= 2:
                        hist["Q1", i, m] = ttr_dot(g1, f"q1_{h}_{i}_{m}")
                    if i >= 3:
                        q2t = ttr_dot(g2, f"q2_{h}_{i}_{m}")

                        P = hist["P", i, m][:]
                        Q1 = hist["Q1", i, m][:]
                        Q2 = q2t[:]
                        S11 = hist["P", i - 1, m][:]
                        S12 = hist["Q1", i - 1, m][:]
                        S22 = hist["P", i - 2, m][:]

                        def tnew(nm):
                            return dots.tile([128, 1], F32, tag="d",
                                             name=f"{nm}_{h}_{i}_{m}")[:]

                        def ts(out, in0, s1, s2, op0, op1=None):
                            nc.vector.tensor_scalar(out, in0, s1, s2, op0,
                                                    *( [op1] if op1 else []))

                        def aff(out, in_, scale, bias):
                            nc.scalar.activation(out, in_, ACTF.Identity,
                                                 bias=bias, scale=scale)

                        r0 = tnew("r0"); ts(r0, P, Q1, None, ALU.subtract)
                        r1 = tnew("r1"); ts(r1, P, Q2, None, ALU.subtract)
                        a1 = tnew("a1"); aff(a1, Q1, -2.0, S11)
                        av = tnew("av"); ts(av, a1, LAM, P, ALU.add, ALU.add)
                        d1 = tnew("d1"); aff(d1, Q2, -2.0, S22)
                        dv = tnew("dv"); ts(dv, d1, LAM, P, ALU.add, ALU.add)
                        b1 = tnew("b1"); aff(b1, Q2, -1.0, S12)
                        bv = tnew("bv"); ts(bv, b1, r0, None, ALU.add)
                        t4 = tnew("t4"); aff(t4, av, dv, 0.0)
                        t5 = tnew("t5"); nc.scalar.square(t5, bv)
                        det = tnew("det")
                        ts(det, t4, 1e-8, t5, ALU.add, ALU.subtract)
                        idet = tnew("idet"); nc.vector.reciprocal(idet, det)
                        g1a = tnew("g1a"); aff(g1a, dv, r0, 0.0)
                        g1b = tnew("g1b"); ts(g1b, bv, r1, None, ALU.mult)
                        g1c = tnew("g1c"); ts(g1c, g1a, g1b, None, ALU.subtract)
                        gam1 = tnew("gam1"); ts(gam1, g1c, idet, None, ALU.mult)
                        g2a = tnew("g2a"); aff(g2a, av, r1, 0.0)
                        g2b = tnew("g2b"); ts(g2b, bv, r0, None, ALU.mult)
                        g2c = tnew("g2c"); ts(g2c, g2a, g2b, None, ALU.subtract)
                        gam2 = tnew("gam2"); ts(gam2, g2c, idet, None, ALU.mult)
                        s0a = tnew("s0a")
                        ts(s0a, gam1, -1.0, gam2, ALU.mult, ALU.subtract)
                        s0 = tnew("s0"); aff(s0, s0a, 1.0, 1.0)

                        # z' = s0*u + gam1*u1 + gam2*u2 (u2 slot is scratch)
                        ju = (i - 2) % 3
                        nc.scalar.mul(uw(ju, u2[m][:]), u2[m][:], gam2)
                        stt(uw(ju, u2[m][:]), u1[m][:], gam1, u2[m][:],
                            ALU.mult, ALU.add)
                        stt(zbuf[m][:], ui[m][:], s0, u2[m][:],
                            ALU.mult, ALU.add)

            for m in range(MT):
                q = h * MT + m
                nc.sync.dma_start(out_d[q * 128:(q + 1) * 128, :], zbuf[m][:])

        emit_half(0)
        if NHALVES > 1:
            emit_half(1)

    nc.compile()
    return nc


def kernel(x_input, W_z, W_x, b):
    x_input = np.ascontiguousarray(x_input, dtype=np.float32)
    W_z = np.ascontiguousarray(W_z, dtype=np.float32)
    W_x = np.ascontiguousarray(W_x, dtype=np.float32)
    b = np.ascontiguousarray(b, dtype=np.float32)

    if "nc" not in _CACHE:
        _CACHE["nc"] = _build()
    nc = _CACHE["nc"]

    in_maps = [{
        "x": x_input[i * BC:(i + 1) * BC],
        "W_z": W_z, "W_x": W_x, "b": b,
    } for i in range(NCORES)]

    run_kw = {}
    if os.environ.get("K_TRACE", "0") == "1":
        run_kw["trace"] = True
        td = os.environ.get("K_TRACE_DIR")
        if td:
            os.makedirs(td, exist_ok=True)
            run_kw["tmpdir"] = td
    res = bass_utils.run_bass_kernel_spmd(nc, in_maps,
                                          core_ids=list(range(NCORES)),
                                          **run_kw)
    _CACHE["res"] = res
    out = np.concatenate([res.results[i]["z_out"] for i in range(NCORES)],
                         axis=0)
    return out.astype(np.float32)



# revision 7
# speedup vs baseline: 1.4468x; 1.4468x over previous
"""DEQ block (Anderson acceleration, 6 iters, m=3) on 8 trn2 NeuronCores.

Data-parallel over batch: each core gets 512 of 4096 samples; W_z/W_x/b
replicated.  Per core the 512 samples are processed as two sequential
halves of 256 (2 m-tiles of 128) so all per-sample state stays SBUF
resident.  v2 changes vs v1:

  - All PE work in 16-bit or fp8: W_z/W_x/x cast to fp16 on load; z/g/u
    state stored fp16 (error stays relative); transposes run fp16
    (1 cycle/row vs fp32's 2, and fp16 LDWEIGHTS is 3.4x cheaper).
  - W_z is pre-scaled by 8 so its fp8(e4m3) image avoids the subnormal
    range; the 1/8 descale rides the ACT tanh `scale` operand.  xwx is
    stored pre-scaled by 8 so the identity-matmul add stays consistent.
  - Iterations 1..NFP8 run the z@W_z matmul in fp8e4 DoubleRow perf mode
    (2 k-planes per instruction, 0.5 cycles/row): z is quantized to fp8
    during the transpose PSUM->SBUF copy, W_z8 packed [128,2,2048].
  - tanh lands in an fp32 scratch tile; g = f - z is computed by DVE from
    that (keeps g's error relative even when g is small late).

Numerics (fp64 model of this exact pipeline vs reference):
  NFP8=3 -> 2.2e-3, NFP8=4 -> 6.9e-3  (gate is 2e-2).
"""

import sys

sys.path.insert(0, "/opt/trn_rl_repo")

import os
import numpy as np
from contextlib import ExitStack

import concourse.bass as bass
import concourse.tile as tile
from concourse import bacc, mybir, masks
from concourse import bass_utils

F32 = mybir.dt.float32
F16 = mybir.dt.float16
F8 = mybir.dt.float8e4
ALU = mybir.AluOpType
ACTF = mybir.ActivationFunctionType
DROW = mybir.MatmulPerfMode.DoubleRow

B, D = 4096, 2048
NCORES = 8
BC = B // NCORES          # 512 samples per core
NHALF = 2                 # sequential halves per core
CH = BC // NHALF          # 256 samples per half
MT = CH // 128            # 2 m-tiles per half
KT = D // 128             # 16 k-tiles
KP = KT // 2              # 8 fp8 k-pair tiles
NT = D // 512             # 4 n-slices
MAX_ITER, MAND = 6, 3
BETA, LAM = 0.8, 1e-4
SC, ISC = 8.0, 0.125      # W_z / xwx pre-scale and its inverse

_CACHE = {}

NITER = int(os.environ.get("K_NITER", str(MAX_ITER)))
NFP8 = int(os.environ.get("K_NFP8", "3"))   # iters 1..NFP8 use fp8 DoubleRow
NHALVES = int(os.environ.get("K_NHALVES", "2"))


def _build():
    nc = bacc.Bacc("TRN2", target_bir_lowering=False, debug=False,
                   num_devices=NCORES)

    x_d = nc.dram_tensor("x", [BC, D], F32, kind="ExternalInput").ap()
    wz_d = nc.dram_tensor("W_z", [D, D], F32, kind="ExternalInput").ap()
    wx_d = nc.dram_tensor("W_x", [D, D], F32, kind="ExternalInput").ap()
    b_d = nc.dram_tensor("b", [D], F32, kind="ExternalInput").ap()
    out_d = nc.dram_tensor("z_out", [BC, D], F32, kind="ExternalOutput").ap()
    # staging for half-1's xwx (computed in phase 0, reloaded at half 1)
    xwx1_d = nc.dram_tensor("xwx1_stage", [MT, 128, D], F16, kind="Internal").ap()

    with tile.TileContext(nc) as tc, ExitStack() as ctx:
        # ---------------- pools ----------------
        state = ctx.enter_context(tc.tile_pool(name="state", bufs=1))

        def persist(shape, nm, dt=F16):
            return state.tile(shape, dt, tag=nm, name=nm)

        wz16 = [persist([128, D], f"wz16_{k}") for k in range(KT)]
        wz8 = [persist([128, 2, D], f"wz8_{k}", F8) for k in range(KP)]
        zbuf = [persist([128, D], f"zbuf{m}") for m in range(MT)]
        gsl = [[persist([128, D], f"g{j}_{m}") for m in range(MT)]
               for j in range(3)]
        usl = [[persist([128, D], f"u{j}_{m}") for m in range(MT)]
               for j in range(3)]
        xwx = [persist([128, D], f"xwx{m}") for m in range(MT)]
        ident = persist([128, 128], "ident", F32)

        wpool = ctx.enter_context(tc.tile_pool(name="wstream", bufs=2))
        wtp = ctx.enter_context(tc.tile_pool(name="wt16p", bufs=3))
        bpool = ctx.enter_context(tc.tile_pool(name="bpool", bufs=3))
        zt16p = ctx.enter_context(tc.tile_pool(name="zt16", bufs=33))
        zt8p = ctx.enter_context(tc.tile_pool(name="zt8", bufs=17))
        ftp = ctx.enter_context(tc.tile_pool(name="ftmp", bufs=4))
        dots = ctx.enter_context(tc.tile_pool(name="dots", bufs=40))
        typs = ctx.enter_context(tc.tile_pool(name="tpsum", bufs=3, space="PSUM"))
        yps = ctx.enter_context(tc.tile_pool(name="ypsum", bufs=4, space="PSUM"))

        pdump = state.tile([128, 512], F32, tag="pdump", name="pdump")
        qdump = state.tile([128, 512], F32, tag="qdump", name="qdump")
        masks.make_identity(nc, ident[:])
        identh = state.tile([128, 128], F16, tag="identh", name="identh")
        nc.vector.tensor_copy(identh[:], ident[:])
        ridh = identh[:]

        def stt(out, in0, scalar, in1, op0, op1):
            nc.vector.scalar_tensor_tensor(
                out=out, in0=in0, scalar=scalar, in1=in1, op0=op0, op1=op1)

        # W_z: DMA fp32 rows, round to fp16 with the x8 pre-scale on DVE;
        # fp8 packed copies (k-pair-major) derived from the fp16 tiles.
        for k in range(KT):
            for j in range(2):
                wrow = wpool.tile([128, 1024], F32, tag="w", name=f"wl{k}_{j}")
                nc.sync.dma_start(wrow[:], wz_d[k * 128:(k + 1) * 128,
                                               j * 1024:(j + 1) * 1024])
                nc.vector.tensor_scalar_mul(
                    wz16[k][:, j * 1024:(j + 1) * 1024], wrow[:], SC)
            if k % 2 == 1:
                for j in range(2):
                    nc.vector.tensor_copy(wz8[k // 2][:, j, :],
                                          wz16[k - 1 + j][:])

        # XT backing: 16 transposed-x k-rows [128, 512] (4 q-cols of 128)
        # live inside the not-yet-used u-ring f16 tiles during phase 0.
        def xt_sl(k, q):
            back = [usl[0][0], usl[0][1], usl[1][0], usl[1][1]][k // 4]
            off = (k % 4) * 512 + q * 128
            return back[:, off:off + 128]

        # ---------------- phase 0: xwx for all 4 quarter-tiles ----------------
        # x fp16 cast lands in the not-yet-used g-ring tiles (written first
        # at iters 1/2, long after these transposes complete)
        xq16s = [gsl[1][0], gsl[1][1], gsl[2][0], gsl[2][1]]
        for q in range(4):
            xq16 = xq16s[q]
            for h2 in range(2):
                xst = wpool.tile([128, 1024], F32, tag="w", name=f"xst{q}_{h2}")
                nc.sync.dma_start(xst[:], x_d[q * 128:(q + 1) * 128,
                                               h2 * 1024:(h2 + 1) * 1024])
                nc.vector.tensor_copy(xq16[:, h2 * 1024:(h2 + 1) * 1024],
                                      xst[:])
            for k in range(KT):
                tp = typs.tile([128, 128], F16, tag="tp", name=f"xtp{q}_{k}")
                nc.tensor.transpose(tp[:], xq16[:, k * 128:(k + 1) * 128], ridh)
                nc.scalar.copy(xt_sl(k, q), tp[:])

        b2d = b_d.rearrange("(p n) -> p n", p=1)
        for n in range(NT):
            b1 = bpool.tile([1, 512], F32, tag="b", name=f"b1_{n}")
            nc.sync.dma_start(b1[:], b2d[:, n * 512:(n + 1) * 512])
            b8 = bpool.tile([1, 512], F32, tag="b", name=f"b8_{n}")
            nc.vector.tensor_scalar_mul(b8[:], b1[:], SC)
            bsl = bpool.tile([128, 512], F32, tag="b", name=f"bsl{n}")
            nc.gpsimd.partition_broadcast(bsl[:], b8[:])
            ps = [yps.tile([128, 512], F32, tag="yp", name=f"xwps{n}_{q}")
                  for q in range(4)]
            for k in range(KT):
                wt = wpool.tile([128, 512], F32, tag="w", name=f"wx{n}_{k}")
                nc.sync.dma_start(wt[:], wx_d[k * 128:(k + 1) * 128,
                                              n * 512:(n + 1) * 512])
                wt16 = wtp.tile([128, 512], F16, tag="wt", name=f"wx16_{n}_{k}")
                nc.vector.tensor_copy(wt16[:], wt[:])
                for q in range(4):
                    nc.tensor.matmul(ps[q][:], xt_sl(k, q), wt16[:],
                                     start=(k == 0), stop=(k == KT - 1))
            for q in range(4):
                # xwx stored pre-scaled: 8*(x@Wx) + 8*b
                dst = xwx[q] if q < MT else zbuf[q - MT]
                stt(dst[:, n * 512:(n + 1) * 512], ps[q][:], SC, bsl[:],
                    ALU.mult, ALU.add)
        for m in range(MT):
            nc.sync.dma_start(xwx1_d[m], zbuf[m][:])

        # ---------------- per-half iterations ----------------
        def emit_half(h):
            if h == 1:
                for m in range(MT):
                    nc.sync.dma_start(xwx[m][:], xwx1_d[m])

            hist = {}  # (kind, i, m) -> [128,1] ap

            # iteration 0: z=0 -> g0 = tanh(xwx/8), u0 = beta*g0, z1 = u0
            for m in range(MT):
                nc.scalar.activation(gsl[0][m][:], xwx[m][:], ACTF.Tanh,
                                     scale=ISC)
                nc.vector.tensor_scalar_mul(usl[0][m][:], gsl[0][m][:], BETA)

            for i in range(1, NITER):
                fp8 = i <= NFP8
                gi, ui = gsl[i % 3], usl[i % 3]
                g1, g2 = gsl[(i - 1) % 3], gsl[(i - 2) % 3]
                u1, u2 = usl[(i - 1) % 3], usl[(i - 2) % 3]
                zc = usl[i - 1] if i <= 3 else zbuf  # current z (alias)

                # transpose z into lhsT k-tiles (fp16 -> fp16/fp8)
                zt = {}
                z8t = {}
                for m in range(MT):
                    for k in range(KT):
                        tp = typs.tile([128, 128], F16, tag="tp",
                                       name=f"tp{h}_{i}_{m}_{k}")
                        nc.tensor.transpose(
                            tp[:], zc[m][:, k * 128:(k + 1) * 128], ridh)
                        if fp8:
                            if k % 2 == 0:
                                z8t[m, k // 2] = zt8p.tile(
                                    [128, 2, 128], F8, tag="z8",
                                    name=f"z8_{h}_{i}_{m}_{k // 2}")
                            nc.scalar.copy(z8t[m, k // 2][:, k % 2, :], tp[:])
                        else:
                            zs = zt16p.tile([128, 128], F16, tag="zt",
                                            name=f"zt{h}_{i}_{m}_{k}")
                            nc.scalar.copy(zs[:], tp[:])
                            zt[m, k] = zs

                # matmul + xwx add + tanh, n-slice major
                for n in range(NT):
                    nsl = slice(n * 512, (n + 1) * 512)
                    ps = [yps.tile([128, 512], F32, tag="yp",
                                   name=f"yp{h}_{i}_{n}_{m}")
                          for m in range(MT)]
                    if fp8:
                        for kp in range(KP):
                            wsl = wz8[kp][:, :, nsl]
                            for m in range(MT):
                                nc.tensor.matmul(ps[m][:], z8t[m, kp][:], wsl,
                                                 start=(kp == 0), stop=False,
                                                 perf_mode=DROW)
                    else:
                        for k in range(KT):
                            wsl = wz16[k][:, nsl]
                            for m in range(MT):
                                nc.tensor.matmul(ps[m][:], zt[m, k][:], wsl,
                                                 start=(k == 0), stop=False)
                    for m in range(MT):
                        nc.tensor.matmul(ps[m][:], ridh, xwx[m][:, nsl],
                                         start=False, stop=True)
                        ft = ftp.tile([128, 512], F32, tag="ft",
                                      name=f"ft{h}_{i}_{n}_{m}")
                        nc.scalar.activation(ft[:], ps[m][:], ACTF.Tanh,
                                             scale=ISC)
                        # g = f - z  (f stays fp32 until the subtract)
                        stt(gi[m][:, nsl], ft[:], 1.0, zc[m][:, nsl],
                            ALU.mult, ALU.subtract)

                for m in range(MT):
                    # u = beta*g + z
                    stt(ui[m][:], gi[m][:], BETA, zc[m][:], ALU.mult, ALU.add)

                    # P = <g,g> on ACT (square + accum), dumped to PSUM
                    pc = dots.tile([128, 4], F32, tag="d", name=f"pc{h}_{i}_{m}")
                    for c in range(4):
                        nc.scalar.activation(pdump[:],
                                             gi[m][:, c * 512:(c + 1) * 512],
                                             ACTF.Square,
                                             accum_out=pc[:, c:c + 1])
                    pp = dots.tile([128, 1], F32, tag="d", name=f"p{h}_{i}_{m}")
                    nc.vector.tensor_reduce(pp[:], pc[:], mybir.AxisListType.X,
                                            ALU.add)
                    hist["P", i, m] = pp

                    def ttr_dot(gh, nm):
                        qc = dots.tile([128, 4], F32, tag="d", name=f"{nm}c")
                        for c in range(4):
                            nc.vector.scalar_tensor_tensor(
                                out=qdump[:],
                                in0=gi[m][:, c * 512:(c + 1) * 512],
                                scalar=1.0,
                                in1=gh[m][:, c * 512:(c + 1) * 512],
                                op0=ALU.mult, op1=ALU.mult,
                                accum_out=qc[:, c:c + 1])
                        qq = dots.tile([128, 1], F32, tag="d", name=nm)
                        nc.vector.tensor_reduce(qq[:], qc[:],
                                                mybir.AxisListType.X, ALU.add)
                        return qq

                    if i >= 2:
                        hist["Q1", i, m] = ttr_dot(g1, f"q1_{h}_{i}_{m}")
                    if i >= 3:
                        q2t = ttr_dot(g2, f"q2_{h}_{i}_{m}")

                        P = hist["P", i, m][:]
                        Q1 = hist["Q1", i, m][:]
                        Q2 = q2t[:]
                        S11 = hist["P", i - 1, m][:]
                        S12 = hist["Q1", i - 1, m][:]
                        S22 = hist["P", i - 2, m][:]

                        def tnew(nm):
                            return dots.tile([128, 1], F32, tag="d",
                                             name=f"{nm}_{h}_{i}_{m}")[:]

                        def ts(out, in0, s1, s2, op0, op1=None):
                            nc.vector.tensor_scalar(out, in0, s1, s2, op0,
                                                    *([op1] if op1 else []))

                        def aff(out, in_, scale, bias):
                            nc.scalar.activation(out, in_, ACTF.Identity,
                                                 bias=bias, scale=scale)

                        r0 = tnew("r0"); ts(r0, P, Q1, None, ALU.subtract)
                        r1 = tnew("r1"); ts(r1, P, Q2, None, ALU.subtract)
                        a1 = tnew("a1"); aff(a1, Q1, -2.0, S11)
                        av = tnew("av"); ts(av, a1, LAM, P, ALU.add, ALU.add)
                        d1 = tnew("d1"); aff(d1, Q2, -2.0, S22)
                        dv = tnew("dv"); ts(dv, d1, LAM, P, ALU.add, ALU.add)
                        b1 = tnew("b1"); aff(b1, Q2, -1.0, S12)
                        bv = tnew("bv"); ts(bv, b1, r0, None, ALU.add)
                        t4 = tnew("t4"); aff(t4, av, dv, 0.0)
                        t5 = tnew("t5"); nc.scalar.square(t5, bv)
                        det = tnew("det")
                        ts(det, t4, 1e-8, t5, ALU.add, ALU.subtract)
                        idet = tnew("idet"); nc.vector.reciprocal(idet, det)
                        g1a = tnew("g1a"); aff(g1a, dv, r0, 0.0)
                        g1b = tnew("g1b"); ts(g1b, bv, r1, None, ALU.mult)
                        g1c = tnew("g1c"); ts(g1c, g1a, g1b, None, ALU.subtract)
                        gam1 = tnew("gam1"); ts(gam1, g1c, idet, None, ALU.mult)
                        g2a = tnew("g2a"); aff(g2a, av, r1, 0.0)
                        g2b = tnew("g2b"); ts(g2b, bv, r0, None, ALU.mult)
                        g2c = tnew("g2c"); ts(g2c, g2a, g2b, None, ALU.subtract)
                        gam2 = tnew("gam2"); ts(gam2, g2c, idet, None, ALU.mult)
                        s0a = tnew("s0a")
                        ts(s0a, gam1, -1.0, gam2, ALU.mult, ALU.subtract)
                        s0 = tnew("s0"); aff(s0, s0a, 1.0, 1.0)

                        # z' = s0*u + gam1*u1 + gam2*u2 (u2 slot is scratch)
                        nc.scalar.mul(u2[m][:], u2[m][:], gam2)
                        stt(u2[m][:], u1[m][:], gam1, u2[m][:],
                            ALU.mult, ALU.add)
                        if i == NITER - 1:
                            # final update goes out fp32, staged per n-slice
                            for n in range(NT):
                                nsl = slice(n * 512, (n + 1) * 512)
                                zo = ftp.tile([128, 512], F32, tag="ft",
                                              name=f"zo{h}_{m}_{n}")
                                stt(zo[:], ui[m][:, nsl], s0, u2[m][:, nsl],
                                    ALU.mult, ALU.add)
                                q = h * MT + m
                                nc.sync.dma_start(
                                    out_d[q * 128:(q + 1) * 128, nsl], zo[:])
                        else:
                            stt(zbuf[m][:], ui[m][:], s0, u2[m][:],
                                ALU.mult, ALU.add)

        emit_half(0)
        if NHALVES > 1:
            emit_half(1)

    nc.compile()
    return nc


def kernel(x_input, W_z, W_x, b):
    x_input = np.ascontiguousarray(x_input, dtype=np.float32)
    W_z = np.ascontiguousarray(W_z, dtype=np.float32)
    W_x = np.ascontiguousarray(W_x, dtype=np.float32)
    b = np.ascontiguousarray(b, dtype=np.float32)

    if "nc" not in _CACHE:
        _CACHE["nc"] = _build()
    nc = _CACHE["nc"]

    in_maps = [{
        "x": x_input[i * BC:(i + 1) * BC],
        "W_z": W_z, "W_x": W_x, "b": b,
    } for i in range(NCORES)]

    run_kw = {}
    if os.environ.get("K_TRACE", "0") == "1":
        run_kw["trace"] = True
        td = os.environ.get("K_TRACE_DIR")
        if td:
            os.makedirs(td, exist_ok=True)
            run_kw["tmpdir"] = td
    res = bass_utils.run_bass_kernel_spmd(nc, in_maps,
                                          core_ids=list(range(NCORES)),
                                          **run_kw)
    _CACHE["res"] = res
    out = np.concatenate([res.results[i]["z_out"] for i in range(NCORES)],
                         axis=0)
    return out.astype(np.float32)
